# revision 11
# baseline (speedup 1.0000x reference)
"""AttentionPool Trainium2 kernel (8 NeuronCores, SPMD, no collectives).

Math (exactly equivalent to the reference up to fp reordering):
    w_i   = silu(h_i @ W1 + b1) @ W2          (b2 cancels; see below)
    num_g = sum_{i in g} h_i * exp(w_i)
    den_g = sum_{i in g} exp(w_i)
    out_g = num_g / (den_g + eps * exp(max_j w_j))

The reference computes softmax with a global-max shift and eps in the
denominator; multiplying num/den by exp(M) shows equality, and b2 cancels
everywhere (including the eps term).

Device work per core, software-pipelined over 1536-node groups:
    mm1:  u^T[hid, n]  = W1half^T @ h^T            (PE, 512-col chunks)
    silu: s^T = silu(u^T + b1)                     (ACT, [128,1536] per half)
    mm2:  w[n] = s^T_tile^T @ W2half               (PE, 1-col matmuls)
    exp:  e = silu(w) / (w - silu(w)) == exp(w)    (ACT+DVE, batched 6 groups)
    S:    S[p, c] = e_p * (batch_rel_p == c)       (DVE tensor_tensor)
    seg:  num[feat, g] += hN_tile^T @ S            (PE, accumulated in PSUM)

Emission order per iteration keeps every engine fed: DMA(g), mm1(g),
silu(g), mm2(g-1), [exp batch], seg(g-7).  ACT (the silu floor) is the
bottleneck; PE/DVE/DMA all fit under it.

Host: shards nodes at graph boundaries (512 graphs/core), builds transposed
fp16 copies, runs SPMD, computes den/max/final divide from the returned w.
"""

import math

import ml_dtypes
import numpy as np

NCORES = 8
G_TOTAL = 4096
G_PER_CORE = G_TOTAL // NCORES  # 512
IN_DIM = 128
HID = 256
EPS = 1e-6
GROUP_NODES = 1536
TILE_NODES = 128
TPG = GROUP_NODES // TILE_NODES  # 12
EXPB = 6  # groups per exp/S batch
NUM_BANK_COLS = 512  # one PSUM bank of f32

BF16 = ml_dtypes.bfloat16
FP16 = np.float16


def _tilepart(a, nt):
    return a.reshape(nt, TILE_NODES, IN_DIM).transpose(1, 0, 2).reshape(
        TILE_NODES, nt * IN_DIM
    )


FP8 = ml_dtypes.float8_e4m3


def _split_ht8(hc, nt):
    """hT in fp8 for the DoubleRow gate matmul: per group, two 64-partition
    feature blocks [upper | lower] of GROUP_NODES cols each."""
    hcT = np.ascontiguousarray(hc.T).astype(FP8)  # [128, npad]
    npad = hc.shape[0]
    ng = npad // GROUP_NODES
    out = np.empty((64, ng * 2 * GROUP_NODES), FP8)
    for g in range(ng):
        base = g * 2 * GROUP_NODES
        gsl = slice(g * GROUP_NODES, (g + 1) * GROUP_NODES)
        out[:, base : base + GROUP_NODES] = hcT[0:64, gsl]
        out[:, base + GROUP_NODES : base + 2 * GROUP_NODES] = hcT[64:128, gsl]
    return out


def _build_host_data(h, batch, W1, b1, W2):
    """Shard at graph boundaries; build per-core arrays + global window plan."""
    N = h.shape[0]
    batch = np.asarray(batch).astype(np.int64)
    cnt = np.bincount(batch, minlength=G_TOTAL)
    cum = np.concatenate([[0], np.cumsum(cnt)])
    bounds = [int(cum[G_PER_CORE * c]) for c in range(NCORES + 1)]
    sizes = np.diff(bounds)
    npad = int(math.ceil(max(sizes) / GROUP_NODES) * GROUP_NODES)
    nt = npad // TILE_NODES
    assert nt <= NUM_BANK_COLS, f"w columns {nt} exceed one PSUM bank"

    # Global (core-invariant) window starts: c0[t] = min over cores of the
    # first graph (relative) in tile t; SPAN covers the max extent.
    lo = np.full(nt, 1 << 30, dtype=np.int64)
    hi = np.full(nt, -1, dtype=np.int64)
    grels = []
    for c in range(NCORES):
        n0, n1 = bounds[c], bounds[c + 1]
        grel = batch[n0:n1] - G_PER_CORE * c
        grels.append(grel)
        ntc = (n1 - n0 + TILE_NODES - 1) // TILE_NODES
        for t in range(ntc):
            seg = grel[TILE_NODES * t : TILE_NODES * t + TILE_NODES]
            lo[t] = min(lo[t], int(seg[0]))
            hi[t] = max(hi[t], int(seg[-1]))
    span = 8
    while span < int(max(hi - lo)) + 1:
        span *= 2
    assert span <= 64, f"window span {span} unexpectedly large"
    c0 = np.where(hi >= 0, lo, 0).astype(np.int64)
    c0 = np.minimum(c0, G_PER_CORE - 1)  # clamp (padding tiles)
    # fill padding tiles' c0 with last valid to keep windows sane
    last = 0
    for t in range(nt):
        if hi[t] >= 0:
            last = c0[t]
        else:
            c0[t] = last
    wdt = np.minimum(span, G_PER_CORE - c0).astype(np.int64)  # clip to 512

    per_core = []
    for c in range(NCORES):
        n0, n1 = bounds[c], bounds[c + 1]
        nc_nodes = n1 - n0
        hc = np.empty((npad, IN_DIM), np.float32)
        hc[:nc_nodes] = h[n0:n1]
        hc[nc_nodes:] = h[n0]  # replicate a real node into padding
        brel = np.full(npad, -1000.0, np.float32)
        g = grels[c].astype(np.float32)
        tidx = np.arange(nc_nodes) // TILE_NODES
        brel[:nc_nodes] = g - c0[tidx]
        per_core.append(
            dict(
                hn16=np.ascontiguousarray(_tilepart(hc.astype(FP16), nt)),
                ht8=_split_ht8(hc, nt),
                hmask=np.ascontiguousarray(
                    (
                        brel.reshape(nt, TILE_NODES).T[:, :, None]
                        == np.arange(span, dtype=np.float32)[None, None, :]
                    )
                    .astype(FP16)
                    .reshape(TILE_NODES, nt * span)
                ),
                n_nodes=nc_nodes,
                grel=grels[c],
            )
        )

    # W1 in fp8 DoubleRow layout: [64, 512]; cols hf*256 + i*128 + m hold
    # W1[64*i + k, hf*128 + m] for partition k.
    W1f = np.asarray(W1, np.float32)
    w18 = np.empty((64, 512), FP8)
    for hf in range(2):
        for i in range(2):
            w18[:, hf * 256 + i * 128 : hf * 256 + (i + 1) * 128] = W1f[
                64 * i : 64 * (i + 1), hf * 128 : (hf + 1) * 128
            ].astype(FP8)
    w2b = np.asarray(W2).reshape(HID, 1)
    w2b = np.ascontiguousarray(
        np.stack([w2b[:128, 0], w2b[128:, 0]], axis=1)
    ).astype(FP16)  # [128, 2]
    b1f = np.asarray(b1).reshape(HID)
    b1f = np.ascontiguousarray(
        np.stack([b1f[:128], b1f[128:]], axis=1)
    ).astype(np.float32)  # [128, 2]

    plan = dict(
        npad=npad,
        nt=nt,
        ngroups=npad // GROUP_NODES,
        span=span,
        c0=c0,
        wdt=wdt,
        bounds=bounds,
        w18=w18,
        w2b=w2b,
        b1f=b1f,
    )
    return per_core, plan


def _legalize_waits(j):
    """Split multi-wait instructions: this container's walrus accepts at most
    one sync-wait per engine instruction. Hoist extras onto standalone
    EventSemaphore instructions (the same form raw-bass wait_ge produces)
    inserted immediately before, on the same engine."""
    n = 0
    for f in j["functions"]:
        for b in f["blocks"]:
            out = []
            for inst in b["instructions"]:
                si = inst.get("sync_info")
                ow = (si or {}).get("on_wait") or []
                if len(ow) > 1 and inst.get("opcode") != "EventSemaphore":
                    for w in ow[:-1]:
                        n += 1
                        out.append(
                            {
                                "debug": inst.get("debug", 0),
                                "engine": inst["engine"],
                                "ins": [],
                                "name": f"{inst['name']}_hw{n}",
                                "opcode": "EventSemaphore",
                                "outs": [],
                                "sync_info": {"on_update": [], "on_wait": [w]},
                            }
                        )
                    si["on_wait"] = [ow[-1]]
                out.append(inst)
            b["instructions"] = out
    return j


def _ensure_ntff_hook():
    import sys
    import types

    try:
        from antenv.axon_hooks import get_axon_ntff_profile_hook  # noqa: F401

        return
    except ImportError:
        pass
    from trn_agent_boot.trn_boot import _ntff_profile_via_ctypes

    hook = _ntff_profile_via_ctypes("/opt/axon/libaxon_pjrt.so")
    mod = types.ModuleType("antenv.axon_hooks")
    holder = {"hook": hook}
    mod.get_axon_ntff_profile_hook = lambda: holder["hook"]
    mod.set_axon_ntff_profile_hook = lambda h: holder.update(hook=h)
    import antenv

    antenv.axon_hooks = mod
    sys.modules["antenv.axon_hooks"] = mod


def _patch_serialization(nc):
    import json

    orig = nc.to_json_bytes

    def patched():
        j = json.loads(orig())
        _legalize_waits(j)
        return json.dumps(j).encode()

    nc.to_json_bytes = patched


def _build_program(plan):
    import concourse.bass as bass
    import concourse.mybir as mybir
    import concourse.tile as tile

    npad, nt, ngroups, span = plan["npad"], plan["nt"], plan["ngroups"], plan["span"]
    c0, wdt = plan["c0"], plan["wdt"]
    fp32 = mybir.dt.float32
    fp16 = mybir.dt.float16
    fp8 = mybir.dt.float8e4
    drow = mybir.MatmulPerfMode.DoubleRow

    nc = bass.Bass("TRN2", target_bir_lowering=True, debug=False)

    hn_d = nc.dram_tensor("hn16", [TILE_NODES, npad], fp16, kind="ExternalInput").ap()
    ht_d = nc.dram_tensor("ht8", [64, 2 * npad], fp8, kind="ExternalInput").ap()
    hmask = nc.dram_tensor(
        "hmask", [TILE_NODES, nt * span], fp16, kind="ExternalInput"
    ).ap()
    w1_d = nc.dram_tensor("W1", [64, 512], fp8, kind="ExternalInput").ap()
    w2_d = nc.dram_tensor("W2", [128, 2], fp16, kind="ExternalInput").ap()
    b1_d = nc.dram_tensor("b1", [128, 2], fp32, kind="ExternalInput").ap()
    onum = nc.dram_tensor(
        "onum", [IN_DIM, NUM_BANK_COLS], fp32, kind="ExternalOutput"
    ).ap()
    ow = nc.dram_tensor("ow", [TILE_NODES, nt], fp32, kind="ExternalOutput").ap()

    silu = mybir.ActivationFunctionType.Silu
    mult = mybir.AluOpType.mult
    sub = mybir.AluOpType.subtract

    # batches of EXPB groups for the exp/S stage; split the last batch in two
    # so the pipeline tail (last exp chain + segs) is shorter.
    batches = []
    g0 = 0
    while g0 < ngroups:
        g1 = min(g0 + EXPB, ngroups)
        batches.append((g0, g1))
        g0 = g1
    if batches and batches[-1][1] - batches[-1][0] >= 2:
        a, b = batches.pop()
        m = (a + b + 1) // 2
        batches.append((a, m))
        batches.append((m, b))
    batch_of = {}
    for bi, (a, b) in enumerate(batches):
        for g in range(a, b):
            batch_of[g] = bi
    # numerator columns below this are final once all groups < split_group
    # have been segment-accumulated (c0 is nondecreasing).
    split_group = batches[-2][0] if len(batches) >= 2 else ngroups
    col_split = int(c0[split_group * TPG]) if split_group < ngroups else 0

    LAG_SEG = EXPB + 1

    with tile.TileContext(nc) as tc:
        with (
            tc.tile_pool(name="consts", bufs=1) as consts,
            tc.tile_pool(name="io", bufs=10) as io,
            tc.tile_pool(name="smat", bufs=3) as smat,
            tc.tile_pool(name="little", bufs=3) as little,
            tc.tile_pool(name="upsum", bufs=1, space="PSUM") as upsum,
            tc.tile_pool(name="wpsum", bufs=1, space="PSUM") as wpsum,
            tc.tile_pool(name="npsum", bufs=1, space="PSUM") as npsum,
        ):
            w1_sb = consts.tile([64, 512], fp8)
            nc.sync.dma_start(w1_sb[:], w1_d[:])
            w2_sb = consts.tile([128, 2], fp16)
            nc.sync.dma_start(w2_sb[:], w2_d[:])
            b1_sb = consts.tile([128, 2], fp32)
            nc.sync.dma_start(b1_sb[:], b1_d[:])
            mask_sb = consts.tile([TILE_NODES, nt * span], fp16)

            # Pre-touch constants on their consuming engines so later ops
            # need only a single-engine sync wait (ISA wait-slot limits).
            pre = consts.tile([TILE_NODES, 2], fp32)
            nc.scalar.copy(pre[:, 0:1], b1_sb[:, 0:1])

            wall_sb = consts.tile([TILE_NODES, nt], fp32)
            w_ps = wpsum.tile([TILE_NODES, NUM_BANK_COLS], fp32)
            num_ps = npsum.tile([IN_DIM, NUM_BANK_COLS], fp32)

            hcomb_tiles = {}
            sa_tiles = {}
            sb_tiles = {}
            s_tiles = {}

            first_w = [True]
            first_seg = [True]

            def w1_ap(hf):
                base = w1_sb[:]
                return bass.AP(
                    base.tensor, base.offset + hf * 256,
                    [base.ap[0], [128, 2], [1, 128]],
                )

            def emit_main(g):
                hn_t = io.tile([TILE_NODES, GROUP_NODES], fp16, tag="hn")
                hcomb_tiles[g] = hn_t
                nc.sync.dma_start(
                    hn_t[:], hn_d[:, g * GROUP_NODES : (g + 1) * GROUP_NODES]
                )
                ht_t = io.tile([64, 2 * GROUP_NODES], fp8, tag="ht")
                nc.sync.dma_start(
                    ht_t[:],
                    ht_d[:, g * 2 * GROUP_NODES : (g + 1) * 2 * GROUP_NODES],
                )

                # mm1: u^T halves via fp8 DoubleRow (K = 2 x 64 feature blocks),
                # 512-node chunks (one PSUM bank each)
                ua = upsum.tile([128, GROUP_NODES], fp32, tag="ua")
                ub = upsum.tile([128, GROUP_NODES], fp32, tag="ub")
                hb = ht_t[:]
                for u_ps, hf in ((ua, 0), (ub, 1)):
                    for ch in range(TPG // 4):
                        rhs = bass.AP(
                            hb.tensor, hb.offset + ch * 512,
                            [hb.ap[0], [GROUP_NODES, 2], [1, 512]],
                        )
                        nc.tensor.matmul(
                            u_ps[:, ch * 512 : (ch + 1) * 512], w1_ap(hf), rhs,
                            start=True, stop=True, perf_mode=drow,
                        )

                # silu -> s^T halves (fp16, SBUF)
                sa = smat.tile([128, GROUP_NODES], fp16, tag="sa")
                nc.scalar.activation(sa[:], ua[:], silu, bias=b1_sb[:, 0:1])
                sb_ = smat.tile([128, GROUP_NODES], fp16, tag="sb")
                nc.scalar.activation(sb_[:], ub[:], silu, bias=b1_sb[:, 1:2])
                sa_tiles[g] = sa
                sb_tiles[g] = sb_

            def emit_mm2(j):
                sa, sb_ = sa_tiles.pop(j), sb_tiles.pop(j)
                for tt in range(TPG):
                    wc = j * TPG + tt
                    ssl = slice(tt * TILE_NODES, (tt + 1) * TILE_NODES)
                    nc.tensor.matmul(
                        w_ps[:, wc : wc + 1], sa[:, ssl], w2_sb[:, 0:1],
                        start=first_w[0], stop=False,
                    )
                    first_w[0] = False
                    nc.tensor.matmul(
                        w_ps[:, wc : wc + 1], sb_[:, ssl], w2_sb[:, 1:2],
                        start=False, stop=True,
                    )

            def emit_expbatch(bi):
                a, b = batches[bi]
                ncols = (b - a) * TPG
                csl = slice(a * TPG, a * TPG + ncols)
                # w export + e = silu(w)/(w - silu(w)) = exp(w)
                sw = little.tile([TILE_NODES, ncols], fp32, tag="sw")
                nc.scalar.activation(sw[:], w_ps[:, csl], silu)
                nc.vector.tensor_copy(wall_sb[:, csl], w_ps[:, csl])
                d_ = little.tile([TILE_NODES, ncols], fp32, tag="d")
                nc.vector.tensor_tensor(d_[:], w_ps[:, csl], sw[:], sub)
                r_ = little.tile([TILE_NODES, ncols], fp32, tag="r")
                nc.vector.reciprocal(r_[:], d_[:])
                e_ = little.tile([TILE_NODES, ncols], fp32, tag="e")
                nc.vector.tensor_mul(e_[:], sw[:], r_[:])

                s_sb = smat.tile([TILE_NODES, ncols * span], fp16, tag="S")
                e_ap = bass.AP(
                    e_[:].tensor, e_[:].offset,
                    [e_[:].ap[0], [1, ncols], [0, span]],
                )
                msl = mask_sb[:, a * TPG * span : (a * TPG + ncols) * span]
                nc.vector.tensor_tensor(s_sb[:], msl, e_ap, mult)
                s_tiles[bi] = s_sb

            def emit_seg(s):
                bi = batch_of[s]
                a, _ = batches[bi]
                s_sb = s_tiles[bi]
                hN_t = hcomb_tiles.pop(s)[:, 0:GROUP_NODES]
                for tt in range(TPG):
                    t = s * TPG + tt
                    col0, width = int(c0[t]), int(wdt[t])
                    fsl = slice(tt * IN_DIM, (tt + 1) * IN_DIM)
                    soff = ((s - a) * TPG + tt) * span
                    ssl2 = slice(soff, soff + width)
                    ncol = slice(col0, col0 + width)
                    nc.tensor.matmul(
                        num_ps[:, ncol], hN_t[:, fsl], s_sb[:, ssl2],
                        start=first_seg[0], stop=False,
                    )
                    first_seg[0] = False

            num_sb = consts.tile([IN_DIM, NUM_BANK_COLS], fp32)
            for it in range(ngroups + LAG_SEG):
                if it < ngroups:
                    emit_main(it)
                if it == 1:
                    # masks are first needed at the batch-0 exp stage; loading
                    # them after group 0/1 keeps the first mm1 off the
                    # critical path.
                    nc.sync.dma_start(mask_sb[:], hmask[:])
                    preb = consts.tile([TILE_NODES, 1], fp32)
                    nc.vector.tensor_copy(preb[:], mask_sb[:, 0:1])
                j = it - 1
                if 0 <= j < ngroups:
                    emit_mm2(j)
                    bi = batch_of[j]
                    if j == batches[bi][1] - 1:
                        emit_expbatch(bi)
                s = it - LAG_SEG
                if 0 <= s < ngroups:
                    emit_seg(s)
                    if s == split_group - 1 and col_split > 0:
                        # numerator cols below col_split are final; drain them
                        # while the last two batches finish.
                        nc.vector.tensor_copy(
                            num_sb[:, :col_split], num_ps[:, :col_split]
                        )
                        nc.sync.dma_start(
                            onum[:, :col_split], num_sb[:, :col_split]
                        )

            nc.sync.dma_start(ow[:], wall_sb[:])
            nc.vector.tensor_copy(num_sb[:, col_split:], num_ps[:, col_split:])
            nc.sync.dma_start(onum[:, col_split:], num_sb[:, col_split:])

    return nc


def kernel(h, batch, W1, b1, W2, b2):
    h = np.asarray(h, dtype=np.float32)
    batch = np.asarray(batch)
    W1 = np.asarray(W1, dtype=np.float32)
    b1 = np.asarray(b1, dtype=np.float32)
    W2 = np.asarray(W2, dtype=np.float32)
    b2 = np.asarray(b2, dtype=np.float32)

    per_core, plan = _build_host_data(h, batch, W1, b1, W2)
    nc = _build_program(plan)

    from concourse.bass_utils import run_bass_kernel_spmd

    in_maps = []
    for c in range(NCORES):
        pc = per_core[c]
        in_maps.append(
            {
                "hn16": pc["hn16"],
                "ht8": pc["ht8"],
                "hmask": pc["hmask"],
                "W1": plan["w18"],
                "W2": plan["w2b"],
                "b1": plan["b1f"],
            }
        )
    _patch_serialization(nc)
    import os
    import time as _time
    trace = bool(os.environ.get("ATT_TRACE"))
    res = None
    if trace:
        # NTFF profile of device 0; the gauge post-processing in this
        # container lacks some tools, so parse the raw ntff json ourselves.
        import glob
        import json as _json
        import tempfile

        _ensure_ntff_hook()
        import concourse.bass_utils as _bu

        _bu.upload_artifacts = lambda d: d  # no bucket in this container
        tdir = os.environ.get("ATT_TRACE_DIR") or tempfile.mkdtemp()
        try:
            res = run_bass_kernel_spmd(
                nc, in_maps, list(range(NCORES)), trace=True, tmpdir=tdir
            )
        except Exception:
            res = None  # post-processing crash; ntff json may still exist
        for f in sorted(glob.glob(os.path.join(tdir, "ntff_*.json"))):
            try:
                s = _json.load(open(f))["summary"]
                if isinstance(s, list):
                    s = s[0]
                print(f"HW exec time: {s['total_time'] * 1e9:.0f} ns")
                break
            except Exception:
                pass
    if res is None:
        res = run_bass_kernel_spmd(nc, in_maps, list(range(NCORES)))
    nbench = int(os.environ.get("ATT_BENCH", "0"))
    if nbench:
        times = []
        for _ in range(nbench):
            t0 = _time.perf_counter()
            res = run_bass_kernel_spmd(nc, in_maps, list(range(NCORES)))
            times.append(_time.perf_counter() - t0)
        best = min(times)
        print(f"exec wall (best of {nbench}): {best*1e3:.2f} ms  "
              f"(times: {[f'{t*1e3:.1f}' for t in times]})")

    # Host: den from w, global max, final divide, assemble.
    out = np.empty((G_TOTAL, IN_DIM), np.float32)
    m_glob = -np.inf
    core_data = []
    for c in range(NCORES):
        r = res.results[c]
        w_flat = np.asarray(r["ow"]).T.reshape(-1)[: per_core[c]["n_nodes"]]
        m_glob = max(m_glob, float(w_flat.max()))
        core_data.append((np.asarray(r["onum"]), w_flat))
    for c in range(NCORES):
        onum, w_flat = core_data[c]
        e = np.exp(w_flat.astype(np.float64))
        den = np.bincount(
            per_core[c]["grel"], weights=e, minlength=G_PER_CORE
        )[:G_PER_CORE]
        den = den + EPS * math.exp(m_glob)
        out[c * G_PER_CORE : (c + 1) * G_PER_CORE] = (
            onum[:, :G_PER_CORE] / den[None, :].astype(np.float32)
        ).T
    return out


# revision 20
# speedup vs baseline: 1.1893x; 1.1893x over previous
"""AttentionPool Trainium2 kernel (8 NeuronCores, SPMD, no collectives).

Math (exactly equivalent to the reference up to fp reordering):
    w_i   = silu(h_i @ W1 + b1) @ W2          (b2 cancels; see below)
    num_g = sum_{i in g} h_i * exp(w_i)
    den_g = sum_{i in g} exp(w_i)
    out_g = num_g / (den_g + eps * exp(max_j w_j))

The reference computes softmax with a global-max shift and eps in the
denominator; multiplying num/den by exp(M) shows equality, and b2 cancels
everywhere (including the eps term).

Device work per core, software-pipelined over 1536-node groups:
    mm1:  u^T[hid, n]  = W1half^T @ h^T            (PE, 512-col chunks)
    silu: s^T = silu(u^T + b1)                     (ACT, [128,1536] per half)
    mm2:  w[n] = s^T_tile^T @ W2half               (PE, 1-col matmuls)
    exp:  e = silu(w) / (w - silu(w)) == exp(w)    (ACT+DVE, batched 6 groups)
    S:    S[p, c] = e_p * (batch_rel_p == c)       (DVE tensor_tensor)
    seg:  num[feat, g] += hN_tile^T @ S            (PE, accumulated in PSUM)

Emission order per iteration keeps every engine fed: DMA(g), mm1(g),
silu(g), mm2(g-1), [exp batch], seg(g-7).  ACT (the silu floor) is the
bottleneck; PE/DVE/DMA all fit under it.

Host: shards nodes at graph boundaries (512 graphs/core), builds transposed
fp16 copies, runs SPMD, computes den/max/final divide from the returned w.
"""

import math

import ml_dtypes
import numpy as np

NCORES = 8
G_TOTAL = 4096
G_PER_CORE = G_TOTAL // NCORES  # 512
IN_DIM = 128
HID = 256
EPS = 1e-6
GROUP_NODES = 1536
TILE_NODES = 128
TPG = GROUP_NODES // TILE_NODES  # 12
EXPB = 6  # groups per exp/S batch
NUM_BANK_COLS = 512  # one PSUM bank of f32

BF16 = ml_dtypes.bfloat16
FP16 = np.float16


def _tilepart(a, nt):
    return a.reshape(nt, TILE_NODES, IN_DIM).transpose(1, 0, 2).reshape(
        TILE_NODES, nt * IN_DIM
    )


def _combine(hc, nt):
    """One fp16 array holding, per group, two GROUP_NODES-col blocks:
    [hN (tile-partitioned) | hT] — a single contiguous group DMA."""
    hn = _tilepart(hc.astype(FP16), nt)
    hT = np.ascontiguousarray(hc.T).astype(FP16)
    ng = nt // TPG
    out = np.empty((TILE_NODES, ng * 2 * GROUP_NODES), FP16)
    for g in range(ng):
        base = g * 2 * GROUP_NODES
        gsl = slice(g * GROUP_NODES, (g + 1) * GROUP_NODES)
        out[:, base : base + GROUP_NODES] = hn[:, gsl]
        out[:, base + GROUP_NODES : base + 2 * GROUP_NODES] = hT[:, gsl]
    return out


def _build_host_data(h, batch, W1, b1, W2):
    """Shard at graph boundaries; build per-core arrays + global window plan."""
    N = h.shape[0]
    batch = np.asarray(batch).astype(np.int64)
    cnt = np.bincount(batch, minlength=G_TOTAL)
    cum = np.concatenate([[0], np.cumsum(cnt)])
    bounds = [int(cum[G_PER_CORE * c]) for c in range(NCORES + 1)]
    sizes = np.diff(bounds)
    npad = int(math.ceil(max(sizes) / GROUP_NODES) * GROUP_NODES)
    nt = npad // TILE_NODES
    assert nt <= NUM_BANK_COLS, f"w columns {nt} exceed one PSUM bank"

    # Global (core-invariant) window starts: c0[t] = min over cores of the
    # first graph (relative) in tile t; SPAN covers the max extent.
    lo = np.full(nt, 1 << 30, dtype=np.int64)
    hi = np.full(nt, -1, dtype=np.int64)
    grels = []
    for c in range(NCORES):
        n0, n1 = bounds[c], bounds[c + 1]
        grel = batch[n0:n1] - G_PER_CORE * c
        grels.append(grel)
        ntc = (n1 - n0 + TILE_NODES - 1) // TILE_NODES
        for t in range(ntc):
            seg = grel[TILE_NODES * t : TILE_NODES * t + TILE_NODES]
            lo[t] = min(lo[t], int(seg[0]))
            hi[t] = max(hi[t], int(seg[-1]))
    span = 8
    while span < int(max(hi - lo)) + 1:
        span *= 2
    assert span <= 64, f"window span {span} unexpectedly large"
    c0 = np.where(hi >= 0, lo, 0).astype(np.int64)
    c0 = np.minimum(c0, G_PER_CORE - 1)  # clamp (padding tiles)
    # fill padding tiles' c0 with last valid to keep windows sane
    last = 0
    for t in range(nt):
        if hi[t] >= 0:
            last = c0[t]
        else:
            c0[t] = last
    wdt = np.minimum(span, G_PER_CORE - c0).astype(np.int64)  # clip to 512

    per_core = []
    for c in range(NCORES):
        n0, n1 = bounds[c], bounds[c + 1]
        nc_nodes = n1 - n0
        hc = np.empty((npad, IN_DIM), np.float32)
        hc[:nc_nodes] = h[n0:n1]
        hc[nc_nodes:] = h[n0]  # replicate a real node into padding
        brel = np.full(npad, -1000.0, np.float32)
        g = grels[c].astype(np.float32)
        tidx = np.arange(nc_nodes) // TILE_NODES
        brel[:nc_nodes] = g - c0[tidx]
        per_core.append(
            dict(
                hcomb=_combine(hc, nt),
                hmask=np.ascontiguousarray(
                    (
                        brel.reshape(nt, TILE_NODES).T[:, :, None]
                        == np.arange(span, dtype=np.float32)[None, None, :]
                    )
                    .astype(FP16)
                    .reshape(TILE_NODES, nt * span)
                ),
                n_nodes=nc_nodes,
                grel=grels[c],
            )
        )

    w1b = np.asarray(W1).astype(FP16)  # [128, 256]
    w2b = np.asarray(W2).reshape(HID, 1)
    w2b = np.ascontiguousarray(
        np.stack([w2b[:128, 0], w2b[128:, 0]], axis=1)
    ).astype(FP16)  # [128, 2]
    b1f = np.asarray(b1).reshape(HID)
    b1f = np.ascontiguousarray(
        np.stack([b1f[:128], b1f[128:]], axis=1)
    ).astype(np.float32)  # [128, 2]

    plan = dict(
        npad=npad,
        nt=nt,
        ngroups=npad // GROUP_NODES,
        span=span,
        c0=c0,
        wdt=wdt,
        bounds=bounds,
        w1b=w1b,
        w2b=w2b,
        b1f=b1f,
    )
    return per_core, plan


def _legalize_waits(j):
    """Split multi-wait instructions: this container's walrus accepts at most
    one sync-wait per engine instruction. Hoist extras onto standalone
    EventSemaphore instructions (the same form raw-bass wait_ge produces)
    inserted immediately before, on the same engine."""
    n = 0
    for f in j["functions"]:
        for b in f["blocks"]:
            out = []
            for inst in b["instructions"]:
                si = inst.get("sync_info")
                ow = (si or {}).get("on_wait") or []
                if len(ow) > 1 and inst.get("opcode") != "EventSemaphore":
                    for w in ow[:-1]:
                        n += 1
                        out.append(
                            {
                                "debug": inst.get("debug", 0),
                                "engine": inst["engine"],
                                "ins": [],
                                "name": f"{inst['name']}_hw{n}",
                                "opcode": "EventSemaphore",
                                "outs": [],
                                "sync_info": {"on_update": [], "on_wait": [w]},
                            }
                        )
                    si["on_wait"] = [ow[-1]]
                out.append(inst)
            b["instructions"] = out
    return j


def _ensure_ntff_hook():
    import sys
    import types

    try:
        from antenv.axon_hooks import get_axon_ntff_profile_hook  # noqa: F401

        return
    except ImportError:
        pass
    from trn_agent_boot.trn_boot import _ntff_profile_via_ctypes

    hook = _ntff_profile_via_ctypes("/opt/axon/libaxon_pjrt.so")
    mod = types.ModuleType("antenv.axon_hooks")
    holder = {"hook": hook}
    mod.get_axon_ntff_profile_hook = lambda: holder["hook"]
    mod.set_axon_ntff_profile_hook = lambda h: holder.update(hook=h)
    import antenv

    antenv.axon_hooks = mod
    sys.modules["antenv.axon_hooks"] = mod


def _patch_serialization(nc):
    import json

    orig = nc.to_json_bytes

    def patched():
        j = json.loads(orig())
        _legalize_waits(j)
        return json.dumps(j).encode()

    nc.to_json_bytes = patched


def _build_program(plan):
    import concourse.bass as bass
    import concourse.mybir as mybir
    import concourse.tile as tile

    npad, nt, ngroups, span = plan["npad"], plan["nt"], plan["ngroups"], plan["span"]
    c0, wdt = plan["c0"], plan["wdt"]
    fp32 = mybir.dt.float32
    fp16 = mybir.dt.float16

    nc = bass.Bass("TRN2", target_bir_lowering=True, debug=False)

    hcomb = nc.dram_tensor(
        "hcomb", [TILE_NODES, 2 * npad], fp16, kind="ExternalInput"
    ).ap()
    hmask = nc.dram_tensor(
        "hmask", [TILE_NODES, nt * span], fp16, kind="ExternalInput"
    ).ap()
    w1_d = nc.dram_tensor("W1", [IN_DIM, HID], fp16, kind="ExternalInput").ap()
    w2_d = nc.dram_tensor("W2", [128, 2], fp16, kind="ExternalInput").ap()
    b1_d = nc.dram_tensor("b1", [128, 2], fp32, kind="ExternalInput").ap()
    onum = nc.dram_tensor(
        "onum", [IN_DIM, NUM_BANK_COLS], fp32, kind="ExternalOutput"
    ).ap()
    ow = nc.dram_tensor("ow", [TILE_NODES, nt], fp32, kind="ExternalOutput").ap()

    silu = mybir.ActivationFunctionType.Silu
    mult = mybir.AluOpType.mult
    sub = mybir.AluOpType.subtract

    # batches of EXPB groups for the exp/S stage; split the last batch in two
    # so the pipeline tail (last exp chain + segs) is shorter.
    batches = []
    g0 = 0
    while g0 < ngroups:
        g1 = min(g0 + EXPB, ngroups)
        batches.append((g0, g1))
        g0 = g1
    if batches and batches[-1][1] - batches[-1][0] >= 2:
        a, b = batches.pop()
        m = (a + b + 1) // 2
        batches.append((a, m))
        batches.append((m, b))
    batch_of = {}
    for bi, (a, b) in enumerate(batches):
        for g in range(a, b):
            batch_of[g] = bi
    # numerator columns below this are final once all groups < split_group
    # have been segment-accumulated (c0 is nondecreasing).
    split_group = batches[-2][0] if len(batches) >= 2 else ngroups
    col_split = int(c0[split_group * TPG]) if split_group < ngroups else 0

    LAG_SEG = EXPB + 1

    with tile.TileContext(nc) as tc:
        with (
            tc.tile_pool(name="consts", bufs=1) as consts,
            tc.tile_pool(name="io", bufs=10) as io,
            tc.tile_pool(name="smat", bufs=3) as smat,
            tc.tile_pool(name="little", bufs=3) as little,
            tc.tile_pool(name="upsum", bufs=1, space="PSUM") as upsum,
            tc.tile_pool(name="wpsum", bufs=1, space="PSUM") as wpsum,
            tc.tile_pool(name="npsum", bufs=1, space="PSUM") as npsum,
        ):
            w1_sb = consts.tile([IN_DIM, HID], fp16)
            nc.sync.dma_start(w1_sb[:], w1_d[:])
            w2_sb = consts.tile([128, 2], fp16)
            nc.sync.dma_start(w2_sb[:], w2_d[:])
            b1_sb = consts.tile([128, 2], fp32)
            nc.sync.dma_start(b1_sb[:], b1_d[:])
            mask_sb = consts.tile([TILE_NODES, nt * span], fp16)

            # Pre-touch constants on their consuming engines so later ops
            # need only a single-engine sync wait (ISA wait-slot limits).
            pre = consts.tile([TILE_NODES, 2], fp32)
            nc.scalar.copy(pre[:, 0:1], b1_sb[:, 0:1])

            wall_sb = consts.tile([TILE_NODES, nt], fp32)
            w_ps = wpsum.tile([TILE_NODES, NUM_BANK_COLS], fp32)
            num_ps = npsum.tile([IN_DIM, NUM_BANK_COLS], fp32)

            hcomb_tiles = {}
            sa_tiles = {}
            sb_tiles = {}
            s_tiles = {}

            first_w = [True]
            first_seg = [True]

            ub_tiles = {}

            def emit_main_a(g):
                hcomb_t = io.tile([TILE_NODES, 2 * GROUP_NODES], fp16, tag="hc")
                hcomb_tiles[g] = hcomb_t
                nc.sync.dma_start(
                    hcomb_t[:],
                    hcomb[:, g * 2 * GROUP_NODES : (g + 1) * 2 * GROUP_NODES],
                )
                hT_t = hcomb_t[:, GROUP_NODES : 2 * GROUP_NODES]

                # mm1 half A + silu-a; half B is emitted after the seg matmuls
                # so the PE has ready work while silu-b(g-1) drains ub's WAR.
                ua = upsum.tile([128, GROUP_NODES], fp32, tag="ua")
                for ch in range(TPG // 4):
                    csl = slice(ch * 512, (ch + 1) * 512)
                    nc.tensor.matmul(
                        ua[:, csl], w1_sb[:, 0:128], hT_t[:, csl],
                        start=True, stop=True,
                    )
                sa = smat.tile([128, GROUP_NODES], fp16, tag="sa")
                nc.scalar.activation(sa[:], ua[:], silu, bias=b1_sb[:, 0:1])
                sa_tiles[g] = sa

            def emit_main_b(g):
                hT_t = hcomb_tiles[g][:, GROUP_NODES : 2 * GROUP_NODES]
                ub = upsum.tile([128, GROUP_NODES], fp32, tag="ub")
                for ch in range(TPG // 4):
                    csl = slice(ch * 512, (ch + 1) * 512)
                    nc.tensor.matmul(
                        ub[:, csl], w1_sb[:, 128:256], hT_t[:, csl],
                        start=True, stop=True,
                    )
                sb_ = smat.tile([128, GROUP_NODES], fp16, tag="sb")
                nc.scalar.activation(sb_[:], ub[:], silu, bias=b1_sb[:, 1:2])
                sb_tiles[g] = sb_

            def emit_mm2(j):
                sa, sb_ = sa_tiles.pop(j), sb_tiles.pop(j)
                for tt in range(TPG):
                    wc = j * TPG + tt
                    ssl = slice(tt * TILE_NODES, (tt + 1) * TILE_NODES)
                    nc.tensor.matmul(
                        w_ps[:, wc : wc + 1], sa[:, ssl], w2_sb[:, 0:1],
                        start=first_w[0], stop=False,
                    )
                    first_w[0] = False
                    nc.tensor.matmul(
                        w_ps[:, wc : wc + 1], sb_[:, ssl], w2_sb[:, 1:2],
                        start=False, stop=True,
                    )

            def emit_expbatch(bi):
                a, b = batches[bi]
                ncols = (b - a) * TPG
                csl = slice(a * TPG, a * TPG + ncols)
                # w export + e = silu(w)/(w - silu(w)) = exp(w)
                sw = little.tile([TILE_NODES, ncols], fp32, tag="sw")
                nc.scalar.activation(sw[:], w_ps[:, csl], silu)
                nc.vector.tensor_copy(wall_sb[:, csl], w_ps[:, csl])
                d_ = little.tile([TILE_NODES, ncols], fp32, tag="d")
                nc.vector.tensor_tensor(d_[:], w_ps[:, csl], sw[:], sub)
                r_ = little.tile([TILE_NODES, ncols], fp32, tag="r")
                nc.vector.reciprocal(r_[:], d_[:])
                e_ = little.tile([TILE_NODES, ncols], fp32, tag="e")
                nc.vector.tensor_mul(e_[:], sw[:], r_[:])

                s_sb = smat.tile([TILE_NODES, ncols * span], fp16, tag="S")
                e_ap = bass.AP(
                    e_[:].tensor, e_[:].offset,
                    [e_[:].ap[0], [1, ncols], [0, span]],
                )
                msl = mask_sb[:, a * TPG * span : (a * TPG + ncols) * span]
                nc.vector.tensor_tensor(s_sb[:], msl, e_ap, mult)
                s_tiles[bi] = s_sb

            def emit_seg(s):
                bi = batch_of[s]
                a, _ = batches[bi]
                s_sb = s_tiles[bi]
                hN_t = hcomb_tiles.pop(s)[:, 0:GROUP_NODES]
                for tt in range(TPG):
                    t = s * TPG + tt
                    col0, width = int(c0[t]), int(wdt[t])
                    fsl = slice(tt * IN_DIM, (tt + 1) * IN_DIM)
                    soff = ((s - a) * TPG + tt) * span
                    ssl2 = slice(soff, soff + width)
                    ncol = slice(col0, col0 + width)
                    nc.tensor.matmul(
                        num_ps[:, ncol], hN_t[:, fsl], s_sb[:, ssl2],
                        start=first_seg[0], stop=False,
                    )
                    first_seg[0] = False

            num_sb = consts.tile([IN_DIM, NUM_BANK_COLS], fp32)
            for it in range(ngroups + LAG_SEG):
                if it < ngroups:
                    emit_main_a(it)
                if it == 1:
                    # masks are first needed at the batch-0 exp stage; loading
                    # them after group 0/1 keeps the first mm1 off the
                    # critical path.
                    nc.sync.dma_start(mask_sb[:], hmask[:])
                    preb = consts.tile([TILE_NODES, 1], fp32)
                    nc.vector.tensor_copy(preb[:], mask_sb[:, 0:1])
                s = it - LAG_SEG
                if 0 <= s < ngroups:
                    emit_seg(s)
                    if s == split_group - 1 and col_split > 0:
                        # numerator cols below col_split are final; drain them
                        # while the last two batches finish.
                        nc.vector.tensor_copy(
                            num_sb[:, :col_split], num_ps[:, :col_split]
                        )
                        nc.sync.dma_start(
                            onum[:, :col_split], num_sb[:, :col_split]
                        )
                if it < ngroups:
                    emit_main_b(it)
                j = it - 1
                if 0 <= j < ngroups:
                    emit_mm2(j)
                    bi = batch_of[j]
                    if j == batches[bi][1] - 1:
                        emit_expbatch(bi)

            nc.sync.dma_start(ow[:], wall_sb[:])
            nc.vector.tensor_copy(num_sb[:, col_split:], num_ps[:, col_split:])
            nc.sync.dma_start(onum[:, col_split:], num_sb[:, col_split:])

    return nc


def kernel(h, batch, W1, b1, W2, b2):
    h = np.asarray(h, dtype=np.float32)
    batch = np.asarray(batch)
    W1 = np.asarray(W1, dtype=np.float32)
    b1 = np.asarray(b1, dtype=np.float32)
    W2 = np.asarray(W2, dtype=np.float32)
    b2 = np.asarray(b2, dtype=np.float32)

    per_core, plan = _build_host_data(h, batch, W1, b1, W2)
    nc = _build_program(plan)

    from concourse.bass_utils import run_bass_kernel_spmd

    in_maps = []
    for c in range(NCORES):
        pc = per_core[c]
        in_maps.append(
            {
                "hcomb": pc["hcomb"],
                "hmask": pc["hmask"],
                "W1": plan["w1b"],
                "W2": plan["w2b"],
                "b1": plan["b1f"],
            }
        )
    _patch_serialization(nc)
    import os
    import time as _time
    trace = bool(os.environ.get("ATT_TRACE"))
    res = None
    if trace:
        # NTFF profile of device 0; the gauge post-processing in this
        # container lacks some tools, so parse the raw ntff json ourselves.
        import glob
        import json as _json
        import tempfile

        _ensure_ntff_hook()
        import concourse.bass_utils as _bu

        _bu.upload_artifacts = lambda d: d  # no bucket in this container
        tdir = os.environ.get("ATT_TRACE_DIR") or tempfile.mkdtemp()
        try:
            res = run_bass_kernel_spmd(
                nc, in_maps, list(range(NCORES)), trace=True, tmpdir=tdir
            )
        except Exception:
            res = None  # post-processing crash; ntff json may still exist
        for f in sorted(glob.glob(os.path.join(tdir, "ntff_*.json"))):
            try:
                s = _json.load(open(f))["summary"]
                if isinstance(s, list):
                    s = s[0]
                print(f"HW exec time: {s['total_time'] * 1e9:.0f} ns")
                break
            except Exception:
                pass
    if res is None:
        res = run_bass_kernel_spmd(nc, in_maps, list(range(NCORES)))
    nbench = int(os.environ.get("ATT_BENCH", "0"))
    if nbench:
        times = []
        for _ in range(nbench):
            t0 = _time.perf_counter()
            res = run_bass_kernel_spmd(nc, in_maps, list(range(NCORES)))
            times.append(_time.perf_counter() - t0)
        best = min(times)
        print(f"exec wall (best of {nbench}): {best*1e3:.2f} ms  "
              f"(times: {[f'{t*1e3:.1f}' for t in times]})")

    # Host: den from w, global max, final divide, assemble.
    out = np.empty((G_TOTAL, IN_DIM), np.float32)
    m_glob = -np.inf
    core_data = []
    for c in range(NCORES):
        r = res.results[c]
        w_flat = np.asarray(r["ow"]).T.reshape(-1)[: per_core[c]["n_nodes"]]
        m_glob = max(m_glob, float(w_flat.max()))
        core_data.append((np.asarray(r["onum"]), w_flat))
    for c in range(NCORES):
        onum, w_flat = core_data[c]
        e = np.exp(w_flat.astype(np.float64))
        den = np.bincount(
            per_core[c]["grel"], weights=e, minlength=G_PER_CORE
        )[:G_PER_CORE]
        den = den + EPS * math.exp(m_glob)
        out[c * G_PER_CORE : (c + 1) * G_PER_CORE] = (
            onum[:, :G_PER_CORE] / den[None, :].astype(np.float32)
        ).T
    return out
